# revision 29
# baseline (speedup 1.0000x reference)
"""Distributed causal MHA for TRN2 (8 NeuronCores), v5.

Core c: batch c//2, interleaved 256-row query blocks (even/odd positions)
for causal load balance; identical SPMD programs, per-core data only in
xTq/mstart. Per head pair and slot pair: wide bf16 score matmuls (one
PSUM bank), one wide exp on ACT, causal mask via (iota>=mstart)*exp on
DVE for diagonal tiles only, per-slot AV accumulation with a ones-column
in V yielding softmax denominators. The second half of the V projection
is emitted as PE filler inside the first attention pass so the in-order
PE queue never starves while ACT computes exponentials. Normalization is
batched at the end (DMA-packed denominator rows, one DVE reciprocal,
K=1 broadcast matmuls); out-projection consumes the d-major attention
output directly as stationary operands with the bias as a K=1 tile.
"""

import sys

sys.path.insert(0, "/opt/trn_rl_repo")
import numpy as np
import ml_dtypes
import concourse.bass as bass
import concourse.mybir as mybir
import concourse.tile as tile
from concourse.vector_clock import ScopedClock
from concourse.bass_utils import run_bass_kernel_spmd

B, N, DIM = 4, 2048, 1024
HEADS, DH = 16, 64
INNER = HEADS * DH
SCALE = DH ** -0.5
NQ = 1024
CH = 256
NSLOT = 4
F32 = mybir.dt.float32
BF16 = mybir.dt.bfloat16
AF = mybir.ActivationFunctionType
ALU = mybir.AluOpType

LAST_RESULT = None


def _drain_and_barrier_patched(self, tick_clock, wait_clock):
    nop_inst = self.nc.sync.nop(nofuse=True)
    wait_clock.add_sem_waits(nop_inst.ins, ScopedClock({None: tick_clock.global_clock}))
    si = nop_inst.ins.sync_info
    waits = list(si.on_wait or []) if si else []
    if len(waits) > 1:
        nop_inst.ins.sync_info = mybir.SyncInfo(
            on_wait=waits[:1], on_update=list(si.on_update or [])
        )
        for i in range(1, len(waits)):
            extra = self.nc.sync.nop(nofuse=True)
            extra.ins.sync_info = mybir.SyncInfo(on_wait=[waits[i]], on_update=[])
    self.nc.sync.drain()
    self.nc.all_engine_barrier()
    popped = self.nc._tile_sem_poison_stack.pop()
    assert popped is self._sem_poison
    self.nc.clear_and_free_semaphores(list(self.sems.allocated().values()))
    self.nc.all_engine_barrier()


tile.TileContext._drain_and_barrier = _drain_and_barrier_patched


def _split_multi_waits(nc):
    for f in nc.m.functions:
        for bb in f.blocks:
            insts = bb.instructions
            if not any(
                i.sync_info and i.sync_info.on_wait and len(i.sync_info.on_wait) > 1
                for i in insts
            ):
                continue
            new = []
            for inst in insts:
                si = inst.sync_info
                waits = list(si.on_wait) if si and si.on_wait else []
                if len(waits) > 1:
                    for w in waits[:-1]:
                        nop = mybir.InstNoOp(
                            name=nc.get_next_instruction_name(), ins=[], outs=[]
                        )
                        nop.engine = inst.engine
                        nop.sync_info = mybir.SyncInfo(on_wait=[w], on_update=[])
                        new.append(nop)
                    inst.sync_info = mybir.SyncInfo(
                        on_wait=[waits[-1]], on_update=list(si.on_update or [])
                    )
                new.append(inst)
            bb.instructions = new


def build_graph():
    nc = bass.Bass("TRN2", target_bir_lowering=False)

    p_xT = nc.declare_dram_parameter("xT", [DIM, N], BF16, isOutput=False)
    p_xTq = nc.declare_dram_parameter("xTq", [DIM, NQ], BF16, isOutput=False)
    p_wq = nc.declare_dram_parameter("w_q", [DIM, INNER], BF16, isOutput=False)
    p_wkv = nc.declare_dram_parameter("w_kv", [DIM, 2 * INNER], BF16, isOutput=False)
    p_wbo = nc.declare_dram_parameter("wb_out", [INNER + 1, DIM], BF16, isOutput=False)
    p_ms = nc.declare_dram_parameter("mstart", [128, 16], BF16, isOutput=False)
    p_iota = nc.declare_dram_parameter("iota", [128, CH], BF16, isOutput=False)
    p_out = nc.declare_dram_parameter("out", [NQ, DIM], F32, isOutput=True)

    with tile.TileContext(nc) as tc:
        with (
            tc.tile_pool(name="const", bufs=1) as cst,
            tc.tile_pool(name="qt", bufs=1) as qtp,
            tc.tile_pool(name="vsb", bufs=1) as vsp,
            tc.tile_pool(name="ktr", bufs=1) as ktrp,
            tc.tile_pool(name="xt", bufs=1) as xtp,
        ):
            iota = cst.tile([128, CH], BF16, tag="iota", name="iota")
            nc.sync.dma_start(iota[:, :], p_iota[:, :])
            ms = cst.tile([128, 16], BF16, tag="ms", name="ms")
            nc.sync.dma_start(ms[:, :], p_ms[:, :])
            ones64 = cst.tile([1, 64], F32, tag="ones64", name="ones64")
            nc.vector.memset(ones64[:, :], 1.0)
            onesb = cst.tile([1, 1024], BF16, tag="onesb", name="onesb")
            nc.vector.memset(onesb[:, :], 1.0)

            qt = [qtp.tile([128, NQ], BF16, tag=f"qt{i}", name=f"qt{i}") for i in range(8)]
            vsb = [vsp.tile([128, HEADS * (DH + 1)], BF16, tag=f"v{i}", name=f"v{i}") for i in range(16)]
            ktr = [ktrp.tile([128, N], BF16, tag=f"kt{i}", name=f"kt{i}") for i in range(8)]
            xt = [xtp.tile([128, N], BF16, tag=f"xt{i}", name=f"xt{i}") for i in range(8)]
            for i in range(8):
                nc.sync.dma_start(xt[i][:, :], p_xT[i * 128:(i + 1) * 128, :])

            # ---------------- P0a: QT = w_q.T @ xTq ----------------
            with (
                tc.tile_pool(name="xtq", bufs=1) as xtqp,
                tc.tile_pool(name="wqp", bufs=1) as wqp,
                tc.tile_pool(name="ps0", bufs=2, space="PSUM") as ps0,
            ):
                xtq = [xtqp.tile([128, NQ], BF16, tag=f"xtq{i}", name=f"xtq{i}") for i in range(8)]
                for i in range(8):
                    nc.sync.dma_start(xtq[i][:, :], p_xTq[i * 128:(i + 1) * 128, :])
                wq = [wqp.tile([128, INNER], BF16, tag=f"wq{i}", name=f"wq{i}") for i in range(8)]
                for i in range(8):
                    nc.sync.dma_start(wq[i][:, :], p_wq[i * 128:(i + 1) * 128, :])
                for ft in range(8):
                    for tc2 in range(2):
                        pq = ps0.tile([128, 512], F32, name="pq")
                        for kt in range(8):
                            nc.tensor.matmul(
                                pq[:, :],
                                wq[kt][:, ft * 128:(ft + 1) * 128],
                                xtq[kt][:, tc2 * 512:(tc2 + 1) * 512],
                                start=(kt == 0),
                                stop=(kt == 7),
                            )
                        nc.scalar.activation(
                            qt[ft][:, tc2 * 512:(tc2 + 1) * 512], pq[:, :], AF.Copy
                        )

            # ---------------- P0b: KT resident ----------------
            with (
                tc.tile_pool(name="wkp", bufs=3) as wkp,
                tc.tile_pool(name="ps1", bufs=2, space="PSUM") as ps1,
            ):
                for ft in range(8):
                    for tc4 in range(4):
                        pk = ps1.tile([128, 512], F32, tag="pk", name="pk")
                        for kt in range(8):
                            wk = wkp.tile([128, 128], BF16, tag=f"wk{tc4 % 2}", name="wk")
                            nc.sync.dma_start(
                                wk[:, :],
                                p_wkv[kt * 128:(kt + 1) * 128, ft * 128:(ft + 1) * 128],
                            )
                            nc.tensor.matmul(
                                pk[:, :],
                                wk[:, :],
                                xt[kt][:, tc4 * 512:(tc4 + 1) * 512],
                                start=(kt == 0),
                                stop=(kt == 7),
                            )
                        nc.scalar.activation(
                            ktr[ft][:, tc4 * 512:(tc4 + 1) * 512], pk[:, :], AF.Copy
                        )

            # ---------------- P1 + V filler ----------------
            afp = tc.alloc_tile_pool(name="af", bufs=1)
            af = [afp.tile([128, NQ], BF16, tag=f"af{i}", name=f"af{i}") for i in range(8)]
            anum = [afp.tile([128, NQ], F32, tag=f"an{i}", name=f"an{i}") for i in range(8)]
            dens = afp.tile([64, CH], F32, tag="dens", name="dens")
            with (
                tc.tile_pool(name="wvp", bufs=3) as wvp,
                tc.tile_pool(name="psV", bufs=2, space="PSUM") as psV,
                tc.tile_pool(name="work", bufs=3) as wkpool,
                tc.tile_pool(name="psS", bufs=1, space="PSUM") as psS,
                tc.tile_pool(name="psA", bufs=1, space="PSUM") as psA,
            ):
                # V projection emitter: V[tt] token-major with interleaved ones
                # cols. Emitted in bursts so it can interleave with attention.
                def v_tile_prog(tt):
                    for fc in range(2):
                        pv = psV.tile([128, 512], F32, tag="pv", name="pv")
                        for kt in range(8):
                            wv = wvp.tile([128, 512], BF16, tag=f"wv{fc}", name="wv")
                            nc.sync.dma_start(
                                wv[:, :],
                                p_wkv[
                                    kt * 128:(kt + 1) * 128,
                                    INNER + fc * 512:INNER + (fc + 1) * 512,
                                ],
                            )
                            nc.tensor.matmul(
                                pv[:, :],
                                xt[kt][:, tt * 128:(tt + 1) * 128],
                                wv[:, :],
                                start=(kt == 0),
                                stop=(kt == 7),
                            )
                            yield
                        dst = vsb[tt][
                            :, fc * 8 * 65:(fc * 8 + 8) * 65
                        ].rearrange("p (g d) -> p g d", g=8)[:, :, 0:64]
                        src = pv[:, :].rearrange("p (g d) -> p g d", g=8)
                        nc.vector.tensor_copy(dst, src)
                    nc.vector.memset(
                        vsb[tt][:, :].rearrange("p (g d) -> p g d", g=16)[:, :, 64:65],
                        1.0,
                    )

                def v_range_gen(tts):
                    for tt in tts:
                        yield from v_tile_prog(tt)

                # First half of V up front (needed by attention pass g=0)
                for _ in v_range_gen(range(8)):
                    pass

                filler = v_range_gen(range(8, 16))

                def p1_block(hp, g, filler):
                    h0, h1 = 2 * hp, 2 * hp + 1
                    kth = ktr[hp]
                    qtile = qt[hp]
                    slo = 2 * g
                    nv = {
                        (hi, si): psA.tile([65, CH], F32, tag=f"nv{hi}{si}", name=f"nv{hi}{si}")
                        for hi in range(2) for si in range(2)
                    }
                    for jt in range(8 * g + 8):
                        smin = max(slo, jt // 4)
                        width = (slo + 2 - smin) * CH
                        need_mask = (jt // 4 == smin)
                        for hi, off, h in ((0, 0, h0), (1, 64, h1)):
                            stW = psS.tile([128, 512], F32, tag=f"stW{hi}", name=f"stW{hi}")
                            nc.tensor.matmul(
                                stW[:, 0:width],
                                kth[off:off + 64, jt * 128:(jt + 1) * 128],
                                qtile[off:off + 64, smin * CH:smin * CH + width],
                                start=True,
                                stop=True,
                            )
                            eW = wkpool.tile([128, 512], BF16, tag=f"eW{hi}", name=f"eW{hi}")
                            nc.scalar.activation(
                                eW[:, 0:width], stW[:, 0:width], AF.Exp, scale=SCALE
                            )
                            if need_mask:
                                em = wkpool.tile([128, CH], BF16, tag=f"em{hi}", name=f"em{hi}")
                                nc.vector.scalar_tensor_tensor(
                                    em[:, :],
                                    iota[:, :],
                                    ms[:, jt:jt + 1],
                                    eW[:, 0:CH],
                                    ALU.is_ge,
                                    ALU.mult,
                                )
                            for si2 in range(smin, slo + 2):
                                navm = nv[(hi, si2 - slo)]
                                rhs = (
                                    em[:, :]
                                    if (need_mask and si2 == smin)
                                    else eW[:, (si2 - smin) * CH:(si2 - smin + 1) * CH]
                                )
                                nc.tensor.matmul(
                                    navm[:, :],
                                    vsb[jt][:, h * 65:(h + 1) * 65],
                                    rhs,
                                    start=(jt == 0),
                                    stop=(jt == 4 * si2 + 3),
                                )
                        # PE filler: a couple of independent projection matmuls
                        if filler is not None:
                            for _ in range(2):
                                next(filler, None)
                    for hi, off, h in ((0, 0, h0), (1, 64, h1)):
                        for si in range(2):
                            s2 = slo + si
                            navm = nv[(hi, si)]
                            nc.vector.tensor_copy(
                                anum[hp][off:off + 64, s2 * CH:(s2 + 1) * CH],
                                navm[0:64, :],
                            )
                            den0 = wkpool.tile([1, CH], F32, tag="den0", name="den0", bufs=4)
                            nc.vector.tensor_copy(den0[:, :], navm[64:65, :])
                            r = h * 4 + s2
                            nc.sync.dma_start(dens[r:r + 1, :], den0[:, :])

                for hp in range(8):
                    p1_block(hp, 0, filler)
                # drain any remaining filler
                for _ in filler:
                    pass
                for hp in range(8):
                    p1_block(hp, 1, None)

            # ---------------- P2: batched normalization ----------------
            with (
                tc.tile_pool(name="nrm", bufs=1) as nrmp,
                tc.tile_pool(name="psR", bufs=4, space="PSUM") as psR,
            ):
                rd = nrmp.tile([64, CH], F32, tag="rd", name="rd")
                nc.vector.reciprocal(rd[:, :], dens[:, :])
                for h in range(HEADS):
                    off = (h % 2) * 64
                    for s in range(NSLOT):
                        r = h * 4 + s
                        rdr = nrmp.tile([1, CH], F32, tag="rdr", name="rdr", bufs=8)
                        nc.sync.dma_start(rdr[:, :], rd[r:r + 1, :])
                        rb = psR.tile([64, CH], F32, tag="rb", name="rb")
                        nc.tensor.matmul(rb[:, :], ones64[:, :], rdr[:, :], start=True, stop=True)
                        nc.vector.tensor_mul(
                            af[h // 2][off:off + 64, s * CH:(s + 1) * CH],
                            anum[h // 2][off:off + 64, s * CH:(s + 1) * CH],
                            rb[:, :],
                        )

            # ---------------- P3: out-projection ----------------
            with (
                tc.tile_pool(name="wop", bufs=1) as wop,
                tc.tile_pool(name="wbp", bufs=1) as wbp,
                tc.tile_pool(name="ow", bufs=3) as owp,
                tc.tile_pool(name="psO", bufs=4, space="PSUM") as psO,
            ):
                wo = [wop.tile([128, DIM], BF16, tag=f"wo{i}", name=f"wo{i}") for i in range(8)]
                for i in range(8):
                    nc.sync.dma_start(wo[i][:, :], p_wbo[i * 128:(i + 1) * 128, :])
                wbias = wbp.tile([1, DIM], BF16, tag="wbias", name="wbias")
                nc.sync.dma_start(wbias[:, :], p_wbo[INNER:INNER + 1, :])
                for it in range(8):
                    for oc in range(2):
                        po = psO.tile([128, 512], F32, tag="po", name="po")
                        for ft in range(8):
                            nc.tensor.matmul(
                                po[:, :],
                                af[ft][:, it * 128:(it + 1) * 128],
                                wo[ft][:, oc * 512:(oc + 1) * 512],
                                start=(ft == 0),
                                stop=False,
                            )
                        nc.tensor.matmul(
                            po[:, :],
                            onesb[:, it * 128:(it + 1) * 128],
                            wbias[:, oc * 512:(oc + 1) * 512],
                            start=False,
                            stop=True,
                        )
                        ot = owp.tile([128, 512], F32, tag="ot", name="ot")
                        nc.scalar.activation(ot[:, :], po[:, :], AF.Copy)
                        nc.sync.dma_start(
                            p_out[it * 128:(it + 1) * 128, oc * 512:(oc + 1) * 512],
                            ot[:, :],
                        )
            afp.release()

    _split_multi_waits(nc)
    return nc


_GRAPH = None


def _get_graph():
    global _GRAPH
    if _GRAPH is None:
        _GRAPH = build_graph()
    return _GRAPH


def _core_row_blocks(c):
    par = c % 2
    return [2 * s + par for s in range(NSLOT)]


def kernel(x, mask, w_qkv, w_out, b_out):
    global LAST_RESULT
    x = np.asarray(x, dtype=np.float32)
    w_qkv = np.asarray(w_qkv, dtype=np.float32)
    w_out = np.asarray(w_out, dtype=np.float32)
    b_out = np.asarray(b_out, dtype=np.float32)

    nc = _get_graph()

    BF = ml_dtypes.bfloat16
    w_q = np.ascontiguousarray(w_qkv[:, :INNER].astype(BF))
    w_kv = np.ascontiguousarray(w_qkv[:, INNER:].astype(BF))
    wb = np.ascontiguousarray(np.vstack([w_out, b_out[None, :]]).astype(BF))
    iota = np.broadcast_to(np.arange(CH, dtype=np.float32), (128, CH)).astype(BF).copy()

    xT = [np.ascontiguousarray(x[b].T.astype(BF)) for b in range(B)]

    in_maps = []
    p = np.arange(128, dtype=np.float32)
    for c in range(8):
        b = c // 2
        blocks = _core_row_blocks(c)
        rows = np.concatenate([np.arange(pos * CH, (pos + 1) * CH) for pos in blocks])
        xTq = np.ascontiguousarray(x[b][rows].T.astype(BF))
        mstart = np.empty((128, 16), np.float32)
        for jt in range(16):
            ibase = blocks[jt // 4] * CH
            mstart[:, jt] = jt * 128 + p - ibase
        mstart = np.clip(mstart, -512, 512).astype(BF)
        in_maps.append(
            {
                "xT": xT[b],
                "xTq": xTq,
                "w_q": w_q,
                "w_kv": w_kv,
                "wb_out": wb,
                "mstart": mstart,
                "iota": iota,
            }
        )

    res = run_bass_kernel_spmd(nc, in_maps, list(range(8)))
    LAST_RESULT = res

    out = np.empty((B, N, DIM), dtype=np.float32)
    for c in range(8):
        b = c // 2
        r = res.results[c]["out"]
        for s, pos in enumerate(_core_row_blocks(c)):
            out[b, pos * CH:(pos + 1) * CH] = r[s * CH:(s + 1) * CH]
    return out


# revision 30
# speedup vs baseline: 1.1106x; 1.1106x over previous
"""Distributed causal MHA for TRN2 (8 NeuronCores), v5.

Core c: batch c//2, interleaved 256-row query blocks (even/odd positions)
for causal load balance; identical SPMD programs, per-core data only in
xTq/mstart. Per head pair and slot pair: wide bf16 score matmuls (one
PSUM bank), one wide exp on ACT, causal mask via (iota>=mstart)*exp on
DVE for diagonal tiles only, per-slot AV accumulation with a ones-column
in V yielding softmax denominators. The second half of the V projection
is emitted as PE filler inside the first attention pass so the in-order
PE queue never starves while ACT computes exponentials. Normalization is
batched at the end (DMA-packed denominator rows, one DVE reciprocal,
K=1 broadcast matmuls); out-projection consumes the d-major attention
output directly as stationary operands with the bias as a K=1 tile.
"""

import sys

sys.path.insert(0, "/opt/trn_rl_repo")
import numpy as np
import ml_dtypes
import concourse.bass as bass
import concourse.mybir as mybir
import concourse.tile as tile
from concourse.vector_clock import ScopedClock
from concourse.bass_utils import run_bass_kernel_spmd

B, N, DIM = 4, 2048, 1024
HEADS, DH = 16, 64
INNER = HEADS * DH
SCALE = DH ** -0.5
NQ = 1024
CH = 256
NSLOT = 4
F32 = mybir.dt.float32
BF16 = mybir.dt.bfloat16
AF = mybir.ActivationFunctionType
ALU = mybir.AluOpType

LAST_RESULT = None


def _drain_and_barrier_patched(self, tick_clock, wait_clock):
    nop_inst = self.nc.sync.nop(nofuse=True)
    wait_clock.add_sem_waits(nop_inst.ins, ScopedClock({None: tick_clock.global_clock}))
    si = nop_inst.ins.sync_info
    waits = list(si.on_wait or []) if si else []
    if len(waits) > 1:
        nop_inst.ins.sync_info = mybir.SyncInfo(
            on_wait=waits[:1], on_update=list(si.on_update or [])
        )
        for i in range(1, len(waits)):
            extra = self.nc.sync.nop(nofuse=True)
            extra.ins.sync_info = mybir.SyncInfo(on_wait=[waits[i]], on_update=[])
    self.nc.sync.drain()
    self.nc.all_engine_barrier()
    popped = self.nc._tile_sem_poison_stack.pop()
    assert popped is self._sem_poison
    self.nc.clear_and_free_semaphores(list(self.sems.allocated().values()))
    self.nc.all_engine_barrier()


tile.TileContext._drain_and_barrier = _drain_and_barrier_patched


def _split_multi_waits(nc):
    for f in nc.m.functions:
        for bb in f.blocks:
            insts = bb.instructions
            if not any(
                i.sync_info and i.sync_info.on_wait and len(i.sync_info.on_wait) > 1
                for i in insts
            ):
                continue
            new = []
            for inst in insts:
                si = inst.sync_info
                waits = list(si.on_wait) if si and si.on_wait else []
                if len(waits) > 1:
                    for w in waits[:-1]:
                        nop = mybir.InstNoOp(
                            name=nc.get_next_instruction_name(), ins=[], outs=[]
                        )
                        nop.engine = inst.engine
                        nop.sync_info = mybir.SyncInfo(on_wait=[w], on_update=[])
                        new.append(nop)
                    inst.sync_info = mybir.SyncInfo(
                        on_wait=[waits[-1]], on_update=list(si.on_update or [])
                    )
                new.append(inst)
            bb.instructions = new


def build_graph():
    nc = bass.Bass("TRN2", target_bir_lowering=False)

    p_xT = nc.declare_dram_parameter("xT", [DIM, N], BF16, isOutput=False)
    p_xTq = nc.declare_dram_parameter("xTq", [DIM, NQ], BF16, isOutput=False)
    p_wq = nc.declare_dram_parameter("w_q", [DIM, INNER], BF16, isOutput=False)
    p_wkv = nc.declare_dram_parameter("w_kv", [DIM, 2 * INNER], BF16, isOutput=False)
    p_wbo = nc.declare_dram_parameter("wb_out", [INNER + 1, DIM], BF16, isOutput=False)
    p_ms = nc.declare_dram_parameter("mstart", [128, 16], BF16, isOutput=False)
    p_iota = nc.declare_dram_parameter("iota", [128, CH], BF16, isOutput=False)
    p_out = nc.declare_dram_parameter("out", [NQ, DIM], F32, isOutput=True)

    with tile.TileContext(nc) as tc:
        with (
            tc.tile_pool(name="const", bufs=1) as cst,
            tc.tile_pool(name="qt", bufs=1) as qtp,
            tc.tile_pool(name="vsb", bufs=1) as vsp,
            tc.tile_pool(name="ktr", bufs=1) as ktrp,
            tc.tile_pool(name="xt", bufs=1) as xtp,
        ):
            iota = cst.tile([128, CH], BF16, tag="iota", name="iota")
            nc.sync.dma_start(iota[:, :], p_iota[:, :])
            ms = cst.tile([128, 16], BF16, tag="ms", name="ms")
            nc.sync.dma_start(ms[:, :], p_ms[:, :])
            ones64 = cst.tile([1, 64], F32, tag="ones64", name="ones64")
            nc.vector.memset(ones64[:, :], 1.0)
            onesb = cst.tile([1, 1024], BF16, tag="onesb", name="onesb")
            nc.vector.memset(onesb[:, :], 1.0)

            qt = [qtp.tile([128, NQ], BF16, tag=f"qt{i}", name=f"qt{i}") for i in range(8)]
            vsb = [vsp.tile([128, HEADS * (DH + 1)], BF16, tag=f"v{i}", name=f"v{i}") for i in range(16)]
            ktr = [ktrp.tile([128, N], BF16, tag=f"kt{i}", name=f"kt{i}") for i in range(8)]
            xt = [xtp.tile([128, N], BF16, tag=f"xt{i}", name=f"xt{i}") for i in range(8)]
            for i in range(8):
                nc.sync.dma_start(xt[i][:, :], p_xT[i * 128:(i + 1) * 128, :])

            # ---------------- P0a: QT = w_q.T @ xTq ----------------
            with (
                tc.tile_pool(name="xtq", bufs=1) as xtqp,
                tc.tile_pool(name="wqp", bufs=1) as wqp,
                tc.tile_pool(name="ps0", bufs=2, space="PSUM") as ps0,
            ):
                xtq = [xtqp.tile([128, NQ], BF16, tag=f"xtq{i}", name=f"xtq{i}") for i in range(8)]
                for i in range(8):
                    nc.sync.dma_start(xtq[i][:, :], p_xTq[i * 128:(i + 1) * 128, :])
                wq = [wqp.tile([128, INNER], BF16, tag=f"wq{i}", name=f"wq{i}") for i in range(8)]
                for i in range(8):
                    nc.sync.dma_start(wq[i][:, :], p_wq[i * 128:(i + 1) * 128, :])
                for ft in range(8):
                    for tc2 in range(2):
                        pq = ps0.tile([128, 512], F32, name="pq")
                        for kt in range(8):
                            nc.tensor.matmul(
                                pq[:, :],
                                wq[kt][:, ft * 128:(ft + 1) * 128],
                                xtq[kt][:, tc2 * 512:(tc2 + 1) * 512],
                                start=(kt == 0),
                                stop=(kt == 7),
                            )
                        nc.scalar.activation(
                            qt[ft][:, tc2 * 512:(tc2 + 1) * 512], pq[:, :], AF.Copy
                        )

            # ---------------- P0b: KT resident ----------------
            with (
                tc.tile_pool(name="wkp", bufs=3) as wkp,
                tc.tile_pool(name="ps1", bufs=2, space="PSUM") as ps1,
            ):
                for ft in range(8):
                    for tc4 in range(4):
                        pk = ps1.tile([128, 512], F32, tag="pk", name="pk")
                        for kt in range(8):
                            wk = wkp.tile([128, 128], BF16, tag=f"wk{tc4 % 2}", name="wk")
                            nc.sync.dma_start(
                                wk[:, :],
                                p_wkv[kt * 128:(kt + 1) * 128, ft * 128:(ft + 1) * 128],
                            )
                            nc.tensor.matmul(
                                pk[:, :],
                                wk[:, :],
                                xt[kt][:, tc4 * 512:(tc4 + 1) * 512],
                                start=(kt == 0),
                                stop=(kt == 7),
                            )
                        nc.scalar.activation(
                            ktr[ft][:, tc4 * 512:(tc4 + 1) * 512], pk[:, :], AF.Copy
                        )

            # ---------------- P1 + V filler ----------------
            afp = tc.alloc_tile_pool(name="af", bufs=1)
            af = [afp.tile([128, NQ], BF16, tag=f"af{i}", name=f"af{i}") for i in range(8)]
            anum = [afp.tile([128, NQ], F32, tag=f"an{i}", name=f"an{i}") for i in range(8)]
            dens = afp.tile([64, CH], F32, tag="dens", name="dens")
            with (
                tc.tile_pool(name="wvp", bufs=1) as wvp,
                tc.tile_pool(name="psV", bufs=2, space="PSUM") as psV,
                tc.tile_pool(name="work", bufs=3) as wkpool,
                tc.tile_pool(name="psS", bufs=1, space="PSUM") as psS,
                tc.tile_pool(name="psA", bufs=1, space="PSUM") as psA,
            ):
                # V projection emitter: V[tt] token-major with interleaved ones
                # cols. Emitted in bursts so it can interleave with attention.
                wvr = [wvp.tile([128, 2 * 512], BF16, tag=f"wvr{i}", name=f"wvr{i}") for i in range(8)]
                for i in range(8):
                    nc.sync.dma_start(wvr[i][:, :], p_wkv[i * 128:(i + 1) * 128, INNER:])

                def v_tile_prog(tt):
                    for fc in range(2):
                        pv = psV.tile([128, 512], F32, tag="pv", name="pv")
                        for kt in range(8):
                            nc.tensor.matmul(
                                pv[:, :],
                                xt[kt][:, tt * 128:(tt + 1) * 128],
                                wvr[kt][:, fc * 512:(fc + 1) * 512],
                                start=(kt == 0),
                                stop=(kt == 7),
                            )
                            yield
                        dst = vsb[tt][
                            :, fc * 8 * 65:(fc * 8 + 8) * 65
                        ].rearrange("p (g d) -> p g d", g=8)[:, :, 0:64]
                        src = pv[:, :].rearrange("p (g d) -> p g d", g=8)
                        nc.vector.tensor_copy(dst, src)
                    nc.vector.memset(
                        vsb[tt][:, :].rearrange("p (g d) -> p g d", g=16)[:, :, 64:65],
                        1.0,
                    )

                def v_range_gen(tts):
                    for tt in tts:
                        yield from v_tile_prog(tt)

                # First half of V up front (needed by attention pass g=0)
                for _ in v_range_gen(range(8)):
                    pass

                filler = v_range_gen(range(8, 16))

                def p1_block(hp, g, filler):
                    h0, h1 = 2 * hp, 2 * hp + 1
                    kth = ktr[hp]
                    qtile = qt[hp]
                    slo = 2 * g
                    nv = {
                        (hi, si): psA.tile([65, CH], F32, tag=f"nv{hi}{si}", name=f"nv{hi}{si}")
                        for hi in range(2) for si in range(2)
                    }
                    for jt in range(8 * g + 8):
                        smin = max(slo, jt // 4)
                        width = (slo + 2 - smin) * CH
                        need_mask = (jt // 4 == smin)
                        for hi, off, h in ((0, 0, h0), (1, 64, h1)):
                            stW = psS.tile([128, 512], F32, tag=f"stW{hi}", name=f"stW{hi}")
                            nc.tensor.matmul(
                                stW[:, 0:width],
                                kth[off:off + 64, jt * 128:(jt + 1) * 128],
                                qtile[off:off + 64, smin * CH:smin * CH + width],
                                start=True,
                                stop=True,
                            )
                            eW = wkpool.tile([128, 512], BF16, tag=f"eW{hi}", name=f"eW{hi}")
                            nc.scalar.activation(
                                eW[:, 0:width], stW[:, 0:width], AF.Exp, scale=SCALE
                            )
                            if need_mask:
                                em = wkpool.tile([128, CH], BF16, tag=f"em{hi}", name=f"em{hi}")
                                nc.vector.scalar_tensor_tensor(
                                    em[:, :],
                                    iota[:, :],
                                    ms[:, jt:jt + 1],
                                    eW[:, 0:CH],
                                    ALU.is_ge,
                                    ALU.mult,
                                )
                            for si2 in range(smin, slo + 2):
                                navm = nv[(hi, si2 - slo)]
                                rhs = (
                                    em[:, :]
                                    if (need_mask and si2 == smin)
                                    else eW[:, (si2 - smin) * CH:(si2 - smin + 1) * CH]
                                )
                                nc.tensor.matmul(
                                    navm[:, :],
                                    vsb[jt][:, h * 65:(h + 1) * 65],
                                    rhs,
                                    start=(jt == 0),
                                    stop=(jt == 4 * si2 + 3),
                                )
                        # PE filler: a couple of independent projection matmuls
                        if filler is not None:
                            for _ in range(2):
                                next(filler, None)
                    for hi, off, h in ((0, 0, h0), (1, 64, h1)):
                        for si in range(2):
                            s2 = slo + si
                            navm = nv[(hi, si)]
                            nc.vector.tensor_copy(
                                anum[hp][off:off + 64, s2 * CH:(s2 + 1) * CH],
                                navm[0:64, :],
                            )
                            den0 = wkpool.tile([1, CH], F32, tag="den0", name="den0", bufs=4)
                            nc.vector.tensor_copy(den0[:, :], navm[64:65, :])
                            r = h * 4 + s2
                            nc.sync.dma_start(dens[r:r + 1, :], den0[:, :])

                for hp in range(8):
                    p1_block(hp, 0, filler)
                # drain any remaining filler
                for _ in filler:
                    pass
                for hp in range(8):
                    p1_block(hp, 1, None)

            # ---------------- P2: batched normalization ----------------
            with (
                tc.tile_pool(name="nrm", bufs=1) as nrmp,
                tc.tile_pool(name="psR", bufs=4, space="PSUM") as psR,
            ):
                rd = nrmp.tile([64, CH], F32, tag="rd", name="rd")
                nc.vector.reciprocal(rd[:, :], dens[:, :])
                for h in range(HEADS):
                    off = (h % 2) * 64
                    for s in range(NSLOT):
                        r = h * 4 + s
                        rdr = nrmp.tile([1, CH], F32, tag="rdr", name="rdr", bufs=8)
                        nc.sync.dma_start(rdr[:, :], rd[r:r + 1, :])
                        rb = psR.tile([64, CH], F32, tag="rb", name="rb")
                        nc.tensor.matmul(rb[:, :], ones64[:, :], rdr[:, :], start=True, stop=True)
                        nc.vector.tensor_mul(
                            af[h // 2][off:off + 64, s * CH:(s + 1) * CH],
                            anum[h // 2][off:off + 64, s * CH:(s + 1) * CH],
                            rb[:, :],
                        )

            # ---------------- P3: out-projection ----------------
            with (
                tc.tile_pool(name="wop", bufs=1) as wop,
                tc.tile_pool(name="wbp", bufs=1) as wbp,
                tc.tile_pool(name="ow", bufs=3) as owp,
                tc.tile_pool(name="psO", bufs=4, space="PSUM") as psO,
            ):
                wo = [wop.tile([128, DIM], BF16, tag=f"wo{i}", name=f"wo{i}") for i in range(8)]
                for i in range(8):
                    nc.sync.dma_start(wo[i][:, :], p_wbo[i * 128:(i + 1) * 128, :])
                wbias = wbp.tile([1, DIM], BF16, tag="wbias", name="wbias")
                nc.sync.dma_start(wbias[:, :], p_wbo[INNER:INNER + 1, :])
                for it in range(8):
                    for oc in range(2):
                        po = psO.tile([128, 512], F32, tag="po", name="po")
                        for ft in range(8):
                            nc.tensor.matmul(
                                po[:, :],
                                af[ft][:, it * 128:(it + 1) * 128],
                                wo[ft][:, oc * 512:(oc + 1) * 512],
                                start=(ft == 0),
                                stop=False,
                            )
                        nc.tensor.matmul(
                            po[:, :],
                            onesb[:, it * 128:(it + 1) * 128],
                            wbias[:, oc * 512:(oc + 1) * 512],
                            start=False,
                            stop=True,
                        )
                        ot = owp.tile([128, 512], F32, tag="ot", name="ot")
                        nc.scalar.activation(ot[:, :], po[:, :], AF.Copy)
                        nc.sync.dma_start(
                            p_out[it * 128:(it + 1) * 128, oc * 512:(oc + 1) * 512],
                            ot[:, :],
                        )
            afp.release()

    _split_multi_waits(nc)
    return nc


_GRAPH = None


def _get_graph():
    global _GRAPH
    if _GRAPH is None:
        _GRAPH = build_graph()
    return _GRAPH


def _core_row_blocks(c):
    par = c % 2
    return [2 * s + par for s in range(NSLOT)]


def kernel(x, mask, w_qkv, w_out, b_out):
    global LAST_RESULT
    x = np.asarray(x, dtype=np.float32)
    w_qkv = np.asarray(w_qkv, dtype=np.float32)
    w_out = np.asarray(w_out, dtype=np.float32)
    b_out = np.asarray(b_out, dtype=np.float32)

    nc = _get_graph()

    BF = ml_dtypes.bfloat16
    w_q = np.ascontiguousarray(w_qkv[:, :INNER].astype(BF))
    w_kv = np.ascontiguousarray(w_qkv[:, INNER:].astype(BF))
    wb = np.ascontiguousarray(np.vstack([w_out, b_out[None, :]]).astype(BF))
    iota = np.broadcast_to(np.arange(CH, dtype=np.float32), (128, CH)).astype(BF).copy()

    xT = [np.ascontiguousarray(x[b].T.astype(BF)) for b in range(B)]

    in_maps = []
    p = np.arange(128, dtype=np.float32)
    for c in range(8):
        b = c // 2
        blocks = _core_row_blocks(c)
        rows = np.concatenate([np.arange(pos * CH, (pos + 1) * CH) for pos in blocks])
        xTq = np.ascontiguousarray(x[b][rows].T.astype(BF))
        mstart = np.empty((128, 16), np.float32)
        for jt in range(16):
            ibase = blocks[jt // 4] * CH
            mstart[:, jt] = jt * 128 + p - ibase
        mstart = np.clip(mstart, -512, 512).astype(BF)
        in_maps.append(
            {
                "xT": xT[b],
                "xTq": xTq,
                "w_q": w_q,
                "w_kv": w_kv,
                "wb_out": wb,
                "mstart": mstart,
                "iota": iota,
            }
        )

    res = run_bass_kernel_spmd(nc, in_maps, list(range(8)))
    LAST_RESULT = res

    out = np.empty((B, N, DIM), dtype=np.float32)
    for c in range(8):
        b = c // 2
        r = res.results[c]["out"]
        for s, pos in enumerate(_core_row_blocks(c)):
            out[b, pos * CH:(pos + 1) * CH] = r[s * CH:(s + 1) * CH]
    return out


# revision 31
# speedup vs baseline: 1.1426x; 1.0288x over previous
"""Distributed causal MHA for TRN2 (8 NeuronCores), v4.

Core c: batch c//2; 256-row query blocks {even|odd positions} of that
batch (causal balance). Slot s statically needs 4(s+1) key tiles; key
tile jt serves slots >= jt//4, so the score matmul for (head, jt) is ONE
wide MM over all those slots' query columns (N = 256*(4-jt//4) <= 1024
bf16), followed by ONE wide exp on ACT. Only the first 256-col block
(slot jt//4) straddles the diagonal -> in-place (iota >= mstart) * exp
on DVE. AV accumulates per slot into column-packed PSUM banks with a
ones-column in V producing softmax denominators; normalization happens
once at the end (DMA-packed denominator rows -> one DVE reciprocal ->
K=1 broadcast matmuls). bf16 matmuls, fp32 accumulation, max-free
softmax.
"""

import sys

sys.path.insert(0, "/opt/trn_rl_repo")
import numpy as np
import ml_dtypes
import concourse.bass as bass
import concourse.mybir as mybir
import concourse.tile as tile
from concourse.vector_clock import ScopedClock
from concourse.bass_utils import run_bass_kernel_spmd

B, N, DIM = 4, 2048, 1024
HEADS, DH = 16, 64
INNER = HEADS * DH
SCALE = DH ** -0.5
NQ = 1024
CH = 256
NSLOT = 4
F32 = mybir.dt.float32
BF16 = mybir.dt.bfloat16
AF = mybir.ActivationFunctionType
ALU = mybir.AluOpType

LAST_RESULT = None


def _drain_and_barrier_patched(self, tick_clock, wait_clock):
    nop_inst = self.nc.sync.nop(nofuse=True)
    wait_clock.add_sem_waits(nop_inst.ins, ScopedClock({None: tick_clock.global_clock}))
    si = nop_inst.ins.sync_info
    waits = list(si.on_wait or []) if si else []
    if len(waits) > 1:
        nop_inst.ins.sync_info = mybir.SyncInfo(
            on_wait=waits[:1], on_update=list(si.on_update or [])
        )
        for i in range(1, len(waits)):
            extra = self.nc.sync.nop(nofuse=True)
            extra.ins.sync_info = mybir.SyncInfo(on_wait=[waits[i]], on_update=[])
    self.nc.sync.drain()
    self.nc.all_engine_barrier()
    popped = self.nc._tile_sem_poison_stack.pop()
    assert popped is self._sem_poison
    self.nc.clear_and_free_semaphores(list(self.sems.allocated().values()))
    self.nc.all_engine_barrier()


tile.TileContext._drain_and_barrier = _drain_and_barrier_patched


def _split_multi_waits(nc):
    for f in nc.m.functions:
        for bb in f.blocks:
            insts = bb.instructions
            if not any(
                i.sync_info and i.sync_info.on_wait and len(i.sync_info.on_wait) > 1
                for i in insts
            ):
                continue
            new = []
            for inst in insts:
                si = inst.sync_info
                waits = list(si.on_wait) if si and si.on_wait else []
                if len(waits) > 1:
                    for w in waits[:-1]:
                        nop = mybir.InstNoOp(
                            name=nc.get_next_instruction_name(), ins=[], outs=[]
                        )
                        nop.engine = inst.engine
                        nop.sync_info = mybir.SyncInfo(on_wait=[w], on_update=[])
                        new.append(nop)
                    inst.sync_info = mybir.SyncInfo(
                        on_wait=[waits[-1]], on_update=list(si.on_update or [])
                    )
                new.append(inst)
            bb.instructions = new


def build_graph():
    nc = bass.Bass("TRN2", target_bir_lowering=False)

    p_xT = nc.declare_dram_parameter("xT", [DIM, N], BF16, isOutput=False)
    p_xTq = nc.declare_dram_parameter("xTq", [DIM, NQ], BF16, isOutput=False)
    p_wq = nc.declare_dram_parameter("w_q", [DIM, INNER], BF16, isOutput=False)
    p_wkv = nc.declare_dram_parameter("w_kv", [DIM, 2 * INNER], BF16, isOutput=False)
    p_wbo = nc.declare_dram_parameter("wb_out", [INNER + 1, DIM], BF16, isOutput=False)
    p_ms = nc.declare_dram_parameter("mstart", [128, 16], BF16, isOutput=False)
    p_iota = nc.declare_dram_parameter("iota", [128, CH], BF16, isOutput=False)
    p_out = nc.declare_dram_parameter("out", [NQ, DIM], F32, isOutput=True)

    with tile.TileContext(nc) as tc:
        with (
            tc.tile_pool(name="const", bufs=1) as cst,
            tc.tile_pool(name="qt", bufs=1) as qtp,
            tc.tile_pool(name="vsb", bufs=1) as vsp,
            tc.tile_pool(name="ktr", bufs=1) as ktrp,
        ):
            iota = cst.tile([128, CH], BF16, tag="iota", name="iota")
            nc.sync.dma_start(iota[:, :], p_iota[:, :])
            ms = cst.tile([128, 16], BF16, tag="ms", name="ms")
            nc.sync.dma_start(ms[:, :], p_ms[:, :])
            ones64 = cst.tile([1, 64], F32, tag="ones64", name="ones64")
            nc.vector.memset(ones64[:, :], 1.0)
            onesb = cst.tile([1, 1024], BF16, tag="onesb", name="onesb")
            nc.vector.memset(onesb[:, :], 1.0)

            qt = [qtp.tile([128, NQ], BF16, tag=f"qt{i}", name=f"qt{i}") for i in range(8)]
            vsb = [vsp.tile([128, HEADS * (DH + 1)], BF16, tag=f"v{i}", name=f"v{i}") for i in range(16)]
            ktr = [ktrp.tile([128, N], BF16, tag=f"kt{i}", name=f"kt{i}") for i in range(8)]

            # ---------------- P0a: QT = w_q.T @ xTq ----------------
            with (
                tc.tile_pool(name="xtq", bufs=1) as xtqp,
                tc.tile_pool(name="wqp", bufs=1) as wqp,
                tc.tile_pool(name="ps0", bufs=2, space="PSUM") as ps0,
            ):
                xtq = [xtqp.tile([128, NQ], BF16, tag=f"xtq{i}", name=f"xtq{i}") for i in range(8)]
                for i in range(8):
                    nc.sync.dma_start(xtq[i][:, :], p_xTq[i * 128:(i + 1) * 128, :])
                wq = [wqp.tile([128, INNER], BF16, tag=f"wq{i}", name=f"wq{i}") for i in range(8)]
                for i in range(8):
                    nc.sync.dma_start(wq[i][:, :], p_wq[i * 128:(i + 1) * 128, :])
                for ft in range(8):
                    for tc2 in range(2):
                        pq = ps0.tile([128, 512], F32, name="pq")
                        for kt in range(8):
                            nc.tensor.matmul(
                                pq[:, :],
                                wq[kt][:, ft * 128:(ft + 1) * 128],
                                xtq[kt][:, tc2 * 512:(tc2 + 1) * 512],
                                start=(kt == 0),
                                stop=(kt == 7),
                            )
                        nc.vector.tensor_copy(
                            qt[ft][:, tc2 * 512:(tc2 + 1) * 512], pq[:, :]
                        )

            # ---------------- P0b/P0c: KT resident, V token-major ----------------
            with tc.tile_pool(name="xt", bufs=1) as xtp:
                xt = [xtp.tile([128, N], BF16, tag=f"xt{i}", name=f"xt{i}") for i in range(8)]
                for i in range(8):
                    nc.sync.dma_start(xt[i][:, :], p_xT[i * 128:(i + 1) * 128, :])

                with (
                    tc.tile_pool(name="wkp", bufs=3) as wkp,
                    tc.tile_pool(name="ps1", bufs=1, space="PSUM") as ps1,
                ):
                    for ft in range(8):
                        pk = [ps1.tile([128, 512], F32, tag=f"pk{j}", name=f"pk{j}") for j in range(4)]
                        for kt in range(8):
                            wk = wkp.tile([128, 128], BF16, tag="wk", name="wk")
                            nc.sync.dma_start(
                                wk[:, :],
                                p_wkv[kt * 128:(kt + 1) * 128, ft * 128:(ft + 1) * 128],
                            )
                            for tc4 in range(4):
                                nc.tensor.matmul(
                                    pk[tc4][:, :],
                                    wk[:, :],
                                    xt[kt][:, tc4 * 512:(tc4 + 1) * 512],
                                    start=(kt == 0),
                                    stop=(kt == 7),
                                )
                        for tc4 in range(4):
                            nc.vector.tensor_copy(
                                ktr[ft][:, tc4 * 512:(tc4 + 1) * 512], pk[tc4][:, :]
                            )

                with (
                    tc.tile_pool(name="wvp", bufs=3) as wvp,
                    tc.tile_pool(name="ps2", bufs=1, space="PSUM") as ps2,
                ):
                    for tgrp in range(2):
                        for fc in range(2):
                            pv = [ps2.tile([128, 512], F32, tag=f"pv{j}", name=f"pv{j}") for j in range(8)]
                            for kt in range(8):
                                wv = wvp.tile([128, 512], BF16, tag="wv", name="wv")
                                nc.sync.dma_start(
                                    wv[:, :],
                                    p_wkv[
                                        kt * 128:(kt + 1) * 128,
                                        INNER + fc * 512:INNER + (fc + 1) * 512,
                                    ],
                                )
                                for t8 in range(8):
                                    tt = tgrp * 8 + t8
                                    nc.tensor.matmul(
                                        pv[t8][:, :],
                                        xt[kt][:, tt * 128:(tt + 1) * 128],
                                        wv[:, :],
                                        start=(kt == 0),
                                        stop=(kt == 7),
                                    )
                            for t8 in range(8):
                                tt = tgrp * 8 + t8
                                dst = vsb[tt][
                                    :, fc * 8 * 65:(fc * 8 + 8) * 65
                                ].rearrange("p (g d) -> p g d", g=8)[:, :, 0:64]
                                src = pv[t8][:, :].rearrange("p (g d) -> p g d", g=8)
                                nc.vector.tensor_copy(dst, src)
                    for tt in range(16):
                        nc.vector.memset(
                            vsb[tt][:, :].rearrange("p (g d) -> p g d", g=16)[:, :, 64:65],
                            1.0,
                        )

            # ---------------- P1: attention ----------------
            afp = tc.alloc_tile_pool(name="af", bufs=1)
            af = [afp.tile([128, NQ], BF16, tag=f"af{i}", name=f"af{i}") for i in range(8)]
            anum = [afp.tile([128, NQ], F32, tag=f"an{i}", name=f"an{i}") for i in range(8)]
            dens = afp.tile([64, CH], F32, tag="dens", name="dens")
            with (
                tc.tile_pool(name="work", bufs=3) as wkpool,
                tc.tile_pool(name="psS", bufs=2, space="PSUM") as psS,
                tc.tile_pool(name="psA", bufs=1, space="PSUM") as psA,
            ):
                for hp in range(8):
                    h0, h1 = 2 * hp, 2 * hp + 1
                    kth = ktr[hp]
                    qtile = qt[hp]
                    for g in range(2):
                        slo = 2 * g            # slots {slo, slo+1}
                        nv = {
                            (hi, si): psA.tile([65, CH], F32, tag=f"nv{hi}{si}", name=f"nv{hi}{si}")
                            for hi in range(2) for si in range(2)
                        }
                        pend = None  # (jt, smin, need_mask, [rhs tiles per head])
                        for jt in range(8 * g + 8):
                            smin = max(slo, jt // 4)
                            width = (slo + 2 - smin) * CH
                            need_mask = (jt // 4 == smin)
                            rhss = []
                            for hi, off, h in ((0, 0, h0), (1, 64, h1)):
                                stW = psS.tile([128, 512], F32, tag=f"stW{hi}", name=f"stW{hi}")
                                nc.tensor.matmul(
                                    stW[:, 0:width],
                                    kth[off:off + 64, jt * 128:(jt + 1) * 128],
                                    qtile[off:off + 64, smin * CH:smin * CH + width],
                                    start=True,
                                    stop=True,
                                )
                                eW = wkpool.tile([128, 512], BF16, tag=f"eW{hi}", name=f"eW{hi}")
                                nc.scalar.activation(
                                    eW[:, 0:width], stW[:, 0:width], AF.Exp, scale=SCALE
                                )
                                if need_mask:
                                    em = wkpool.tile([128, CH], BF16, tag=f"em{hi}", name=f"em{hi}")
                                    nc.vector.scalar_tensor_tensor(
                                        em[:, :],
                                        iota[:, :],
                                        ms[:, jt:jt + 1],
                                        eW[:, 0:CH],
                                        ALU.is_ge,
                                        ALU.mult,
                                    )
                                else:
                                    em = None
                                rhss.append((em, eW))
                            if pend is not None:
                                pjt, psmin, pmask, prhss = pend
                                for hi, off, h in ((0, 0, h0), (1, 64, h1)):
                                    pem, peW = prhss[hi]
                                    for si2 in range(psmin, slo + 2):
                                        navm = nv[(hi, si2 - slo)]
                                        rhs = (
                                            pem[:, :]
                                            if (pmask and si2 == psmin)
                                            else peW[:, (si2 - psmin) * CH:(si2 - psmin + 1) * CH]
                                        )
                                        nc.tensor.matmul(
                                            navm[:, :],
                                            vsb[pjt][:, h * 65:(h + 1) * 65],
                                            rhs,
                                            start=(pjt == 0),
                                            stop=(pjt == 4 * si2 + 3),
                                        )
                            pend = (jt, smin, need_mask, rhss)
                        # drain last pending AV
                        pjt, psmin, pmask, prhss = pend
                        for hi, off, h in ((0, 0, h0), (1, 64, h1)):
                            pem, peW = prhss[hi]
                            for si2 in range(psmin, slo + 2):
                                navm = nv[(hi, si2 - slo)]
                                rhs = (
                                    pem[:, :]
                                    if (pmask and si2 == psmin)
                                    else peW[:, (si2 - psmin) * CH:(si2 - psmin + 1) * CH]
                                )
                                nc.tensor.matmul(
                                    navm[:, :],
                                    vsb[pjt][:, h * 65:(h + 1) * 65],
                                    rhs,
                                    start=(pjt == 0),
                                    stop=(pjt == 4 * si2 + 3),
                                )
                        for hi, off, h in ((0, 0, h0), (1, 64, h1)):
                            for si in range(2):
                                s2 = slo + si
                                navm = nv[(hi, si)]
                                nc.vector.tensor_copy(
                                    anum[hp][off:off + 64, s2 * CH:(s2 + 1) * CH],
                                    navm[0:64, :],
                                )
                                den0 = wkpool.tile([1, CH], F32, tag="den0", name="den0", bufs=4)
                                nc.vector.tensor_copy(den0[:, :], navm[64:65, :])
                                r = h * 4 + s2
                                nc.sync.dma_start(dens[r:r + 1, :], den0[:, :])

            # ---------------- P2: batched normalization ----------------
            with (
                tc.tile_pool(name="nrm", bufs=1) as nrmp,
                tc.tile_pool(name="psR", bufs=4, space="PSUM") as psR,
            ):
                rd = nrmp.tile([64, CH], F32, tag="rd", name="rd")
                nc.vector.reciprocal(rd[:, :], dens[:, :])
                for h in range(HEADS):
                    off = (h % 2) * 64
                    for s in range(NSLOT):
                        r = h * 4 + s
                        rdr = nrmp.tile([1, CH], F32, tag="rdr", name="rdr", bufs=8)
                        nc.sync.dma_start(rdr[:, :], rd[r:r + 1, :])
                        rb = psR.tile([64, CH], F32, tag="rb", name="rb")
                        nc.tensor.matmul(rb[:, :], ones64[:, :], rdr[:, :], start=True, stop=True)
                        nc.vector.tensor_mul(
                            af[h // 2][off:off + 64, s * CH:(s + 1) * CH],
                            anum[h // 2][off:off + 64, s * CH:(s + 1) * CH],
                            rb[:, :],
                        )

            # ---------------- P3: out-projection ----------------
            with (
                tc.tile_pool(name="wop", bufs=1) as wop,
                tc.tile_pool(name="wbp", bufs=1) as wbp,
                tc.tile_pool(name="ow", bufs=3) as owp,
                tc.tile_pool(name="psO", bufs=4, space="PSUM") as psO,
            ):
                wo = [wop.tile([128, DIM], BF16, tag=f"wo{i}", name=f"wo{i}") for i in range(8)]
                for i in range(8):
                    nc.sync.dma_start(wo[i][:, :], p_wbo[i * 128:(i + 1) * 128, :])
                wbias = wbp.tile([1, DIM], BF16, tag="wbias", name="wbias")
                nc.sync.dma_start(wbias[:, :], p_wbo[INNER:INNER + 1, :])
                for it in range(8):
                    for oc in range(2):
                        po = psO.tile([128, 512], F32, tag="po", name="po")
                        for ft in range(8):
                            nc.tensor.matmul(
                                po[:, :],
                                af[ft][:, it * 128:(it + 1) * 128],
                                wo[ft][:, oc * 512:(oc + 1) * 512],
                                start=(ft == 0),
                                stop=False,
                            )
                        nc.tensor.matmul(
                            po[:, :],
                            onesb[:, it * 128:(it + 1) * 128],
                            wbias[:, oc * 512:(oc + 1) * 512],
                            start=False,
                            stop=True,
                        )
                        ot = owp.tile([128, 512], F32, tag="ot", name="ot")
                        nc.vector.tensor_copy(ot[:, :], po[:, :])
                        nc.sync.dma_start(
                            p_out[it * 128:(it + 1) * 128, oc * 512:(oc + 1) * 512],
                            ot[:, :],
                        )
            afp.release()

    _split_multi_waits(nc)
    return nc


_GRAPH = None


def _get_graph():
    global _GRAPH
    if _GRAPH is None:
        _GRAPH = build_graph()
    return _GRAPH


def _core_row_blocks(c):
    par = c % 2
    return [2 * s + par for s in range(NSLOT)]


def kernel(x, mask, w_qkv, w_out, b_out):
    global LAST_RESULT
    x = np.asarray(x, dtype=np.float32)
    w_qkv = np.asarray(w_qkv, dtype=np.float32)
    w_out = np.asarray(w_out, dtype=np.float32)
    b_out = np.asarray(b_out, dtype=np.float32)

    nc = _get_graph()

    BF = ml_dtypes.bfloat16
    w_q = np.ascontiguousarray(w_qkv[:, :INNER].astype(BF))
    w_kv = np.ascontiguousarray(w_qkv[:, INNER:].astype(BF))
    wb = np.ascontiguousarray(np.vstack([w_out, b_out[None, :]]).astype(BF))
    iota = np.broadcast_to(np.arange(CH, dtype=np.float32), (128, CH)).astype(BF).copy()

    xT = [np.ascontiguousarray(x[b].T.astype(BF)) for b in range(B)]

    in_maps = []
    p = np.arange(128, dtype=np.float32)
    for c in range(8):
        b = c // 2
        blocks = _core_row_blocks(c)
        rows = np.concatenate([np.arange(pos * CH, (pos + 1) * CH) for pos in blocks])
        xTq = np.ascontiguousarray(x[b][rows].T.astype(BF))
        # mstart[:, jt]: causal start for the diagonal block (slot jt//4)
        mstart = np.empty((128, 16), np.float32)
        for jt in range(16):
            ibase = blocks[jt // 4] * CH
            mstart[:, jt] = jt * 128 + p - ibase
        mstart = np.clip(mstart, -512, 512).astype(BF)
        in_maps.append(
            {
                "xT": xT[b],
                "xTq": xTq,
                "w_q": w_q,
                "w_kv": w_kv,
                "wb_out": wb,
                "mstart": mstart,
                "iota": iota,
            }
        )

    res = run_bass_kernel_spmd(nc, in_maps, list(range(8)))
    LAST_RESULT = res

    out = np.empty((B, N, DIM), dtype=np.float32)
    for c in range(8):
        b = c // 2
        r = res.results[c]["out"]
        for s, pos in enumerate(_core_row_blocks(c)):
            out[b, pos * CH:(pos + 1) * CH] = r[s * CH:(s + 1) * CH]
    return out
